# revision 1
# baseline (speedup 1.0000x reference)
"""Trainium2 Bass kernel for nn_DFlashAttentionSlide (GQA attention block).

Sharding: tensor-parallel over heads across 8 NeuronCores. Core c owns
kv head c and q heads [4c, 4c+4). Activations (x/x_ctx) are replicated;
weights / kv-cache are sharded along the head dim; the output projection
is contraction-sharded, so each core returns a partial [L, HID] output
that the host sums.

Device-side layout strategy (per core):
  - projections:  q as [l, hd] (N=512 matmuls), k/v as [d, t] (N=512)
  - attention scores computed TRANSPOSED: scoresT[s, (h l)] = K @ Q^T
    with k tiles as the stationary operand and all 4 heads' q packed in
    the 512-wide moving operand.  The PV matmul consumes the exp tiles
    directly (contraction over s = partition dim) producing outT
    [d, (h l)] -- no probability transposes anywhere.
  - the causal mask is applied MULTIPLICATIVELY after exp: exp(s+m) =
    exp(s)*exp(m), with exp(mask) precomputed on the host and
    head-replicated on device, so the s-loop mask op is a flat bf16 mul.
  - RMSNorm mean-subtract is folded into the projection weights on the
    host; variance uses sum-of-squares via ones-matmul partition
    reductions; rstd broadcast back across partitions with a K=1
    ones-matmul.
  - RoPE rotate-half is a cross-partition move done with two SBUF->SBUF
    DMA copies; the sign flip is folded into host-built sin tables.
    SCALE (1/sqrt(D)) is folded into the q-side cos/sin tables.
  - all HBM traffic runs on the hardware-DGE (sync) ring; resident
    tensors (kv cache halves, mask, tables, Wo) are chunked and
    interleaved into the projection stream so the PE-feeding cT tiles
    keep queue priority.  GPSIMD does elementwise work only.
"""

import os
import sys

sys.path.insert(0, "/opt/trn_rl_repo")

import numpy as np
import ml_dtypes

import concourse.bass as bass
import concourse.bacc as bacc
import concourse.tile as tile
from concourse import mybir
from concourse.bass_utils import run_bass_kernel_spmd

BF16 = ml_dtypes.bfloat16

H, HKV, D, HALF = 32, 8, 128, 64
L, T, S, HID = 128, 1024, 4096, 4096
REP = H // HKV          # q heads per kv head (= per core)
EPS = 1e-6
SCALE = D ** -0.5
NCORES = 8
KT = HID // 128         # 32 contraction tiles for projections
ST = S // 128           # 32 s tiles for attention
SOLD = S - T            # 3072 cached stream positions kept
TNEW = T                # 1024 newly projected stream positions

FP32 = mybir.dt.float32
BF16_DT = mybir.dt.bfloat16

_PROGRAM_CACHE = {}

# Filled by kernel() when BASS_KERNEL_TRACE=1; read by test.py.
LAST_RESULTS = None


def _build_program():
    nc = bacc.Bacc("TRN2", target_bir_lowering=False, debug=False,
                   num_devices=NCORES)

    # ---- external I/O (per-core values supplied via in_maps) ----
    cT = nc.declare_dram_parameter("cT", [HID, T], BF16_DT, isOutput=False)
    wkvT = nc.declare_dram_parameter("wkvT", [HID, 256], BF16_DT, isOutput=False)
    wqT = nc.declare_dram_parameter("wqT", [HID, 512], BF16_DT, isOutput=False)
    xTp = nc.declare_dram_parameter("xTp", [128, KT * 128], BF16_DT, isOutput=False)
    woP = nc.declare_dram_parameter("woP", [128, HID // 512, REP, 512], BF16_DT, isOutput=False)
    ktold = nc.declare_dram_parameter("ktold", [D, SOLD], BF16_DT, isOutput=False)
    voldP = nc.declare_dram_parameter("voldP", [128, SOLD], BF16_DT, isOutput=False)
    identf = nc.declare_dram_parameter("identf", [128, 128], FP32, isOutput=False)
    identb2 = nc.declare_dram_parameter("identb2", [128, 128], BF16_DT, isOutput=False)
    maskT = nc.declare_dram_parameter("maskT", [128, S], BF16_DT, isOutput=False)
    cosq = nc.declare_dram_parameter("cosq", [D, L], FP32, isOutput=False)
    sinq = nc.declare_dram_parameter("sinq", [D, L], FP32, isOutput=False)
    cosk = nc.declare_dram_parameter("cosk", [D, TNEW], FP32, isOutput=False)
    sink = nc.declare_dram_parameter("sink", [D, TNEW], FP32, isOutput=False)
    qw = nc.declare_dram_parameter("qw", [D, 1], FP32, isOutput=False)
    kw = nc.declare_dram_parameter("kw", [D, 1], FP32, isOutput=False)
    y = nc.declare_dram_parameter("y", [L, HID], FP32, isOutput=True)

    with tile.TileContext(nc) as tc:
        _emit(nc, tc, cT=cT, wkvT=wkvT, wqT=wqT, xTp=xTp, woP=woP, ktold=ktold, voldP=voldP,
              identf=identf, identb2=identb2,
              maskT=maskT, cosq=cosq, sinq=sinq, cosk=cosk, sink=sink,
              qw=qw, kw=kw, y=y)
    nc.compile()
    return nc


def _emit(nc, tc, *, cT, wkvT, wqT, xTp, woP, ktold, voldP, identf, identb2,
          maskT, cosq, sinq, cosk, sink, qw, kw, y):
    from contextlib import ExitStack
    from concourse.masks import make_identity

    ctx = ExitStack()
    with ctx:
        # ---------------- pools ----------------
        consts = ctx.enter_context(tc.tile_pool(name="consts", bufs=1))
        streams = ctx.enter_context(tc.tile_pool(name="streams", bufs=1))
        proj_in = ctx.enter_context(tc.tile_pool(name="proj_in", bufs=6))
        normtmp = ctx.enter_context(tc.tile_pool(name="normtmp", bufs=1))
        sloop = ctx.enter_context(tc.tile_pool(name="sloop", bufs=4))
        psA = ctx.enter_context(tc.tile_pool(name="psA", bufs=1, space="PSUM"))
        psS = ctx.enter_context(tc.tile_pool(name="psS", bufs=3, space="PSUM"))

        # ---------------- constants (no DMA) ----------------
        ones_col = consts.tile([128, 1], FP32, tag="ones_col")
        nc.vector.memset(ones_col, 1.0)
        ones_colb = consts.tile([128, 1], BF16_DT, tag="ones_colb")
        nc.vector.memset(ones_colb, 1.0)
        ones_row = consts.tile([1, 128], FP32, tag="ones_row")
        nc.vector.memset(ones_row, 1.0)
        eps_t = consts.tile([128, 1], FP32, tag="eps")
        nc.vector.memset(eps_t, EPS)
        ident = consts.tile([128, 128], FP32, tag="ident")
        nc.sync.dma_start(ident[:], identf[:])
        identb = consts.tile([128, 128], BF16_DT, tag="identb")
        nc.sync.dma_start(identb[:], identb2[:])

        # ---------------- resident tiles (DMAs interleaved below) --------
        kts = streams.tile([128, S], BF16_DT, tag="kts")
        vt = streams.tile([128, S], BF16_DT, tag="vt")
        mask_all = streams.tile([128, S], BF16_DT, tag="mask")
        mask4 = streams.tile([128, ST, REP, 128], BF16_DT, tag="mask4")
        wo_res = streams.tile([128, HID // 512, REP, 512], BF16_DT, tag="wo")
        qw_t = consts.tile([D, 1], FP32, tag="qw")
        kw_t = consts.tile([D, 1], FP32, tag="kw")
        cosq_t = consts.tile([D, L], FP32, tag="cosq")
        sinq_t = consts.tile([D, L], FP32, tag="sinq")
        cosk_t = consts.tile([D, TNEW], FP32, tag="cosk")
        sink_t = consts.tile([D, TNEW], FP32, tag="sink")



        xT_res = streams.tile([128, KT * 128], BF16_DT, tag="xT")

        def resident_chunk(k):
            # early-needed resident loads only (kts/vt/mask/tables), split
            # across the two HWDGE queues; wo loads happen during the s-loop
            if k < 8:  # kts old: 8 x [128, 384] on qSP
                nc.sync.dma_start(kts[:, k * 384:(k + 1) * 384],
                                  ktold[:, k * 384:(k + 1) * 384])
            if 24 <= k < 32:  # x.T for the q projection: 8 x [128, 512]
                j = k - 24
                nc.sync.dma_start(xT_res[:, j * 512:(j + 1) * 512],
                                  xTp[:, j * 512:(j + 1) * 512])
            if k < 24:  # v old (host-packed): 24 x [128, 128] contiguous
                nc.scalar.dma_start(vt[:, k * 128:(k + 1) * 128],
                                    voldP[:, k * 128:(k + 1) * 128])
            if 8 <= k < 16:  # mask: 8 x [128, 512] on qSP
                j = k - 8
                nc.sync.dma_start(mask_all[:, j * 512:(j + 1) * 512],
                                  maskT[:, j * 512:(j + 1) * 512])
            if 16 <= k < 24:  # rope tables + norm weights on qSP
                j = k - 16
                if j < 2:
                    nc.sync.dma_start(cosk_t[:, j * 512:(j + 1) * 512],
                                      cosk[:, j * 512:(j + 1) * 512])
                elif j < 4:
                    jj = j - 2
                    nc.sync.dma_start(sink_t[:, jj * 512:(jj + 1) * 512],
                                      sink[:, jj * 512:(jj + 1) * 512])
                elif j == 4:
                    nc.sync.dma_start(cosq_t[:], cosq[:])
                elif j == 5:
                    nc.sync.dma_start(sinq_t[:], sinq[:])
                elif j == 6:
                    nc.sync.dma_start(qw_t[:], qw[:])
                else:
                    nc.sync.dma_start(kw_t[:], kw[:])

        # ---------------- projections ----------------
        ps_q = psA.tile([128, 512], FP32, tag="ps_q")
        ps_k0 = psA.tile([128, 512], FP32, tag="ps_k0")
        ps_k1 = psA.tile([128, 512], FP32, tag="ps_k1")
        ps_v0 = psA.tile([128, 512], FP32, tag="ps_v0")
        ps_v1 = psA.tile([128, 512], FP32, tag="ps_v1")

        with nc.named_scope("proj"):
            for k in range(KT):
                ct_k = proj_in.tile([128, T], BF16_DT, tag="ct")
                nc.sync.dma_start(ct_k[:], cT[k * 128:(k + 1) * 128, :])
                w_k = proj_in.tile([128, 256], BF16_DT, tag="wkv")
                nc.scalar.dma_start(w_k[:], wkvT[k * 128:(k + 1) * 128, :])
                resident_chunk(k)

                st = (k == 0)
                sp = (k == KT - 1)
                nc.tensor.matmul(ps_k0[:], w_k[:, 0:128], ct_k[:, 0:512],
                                 start=st, stop=sp)
                nc.tensor.matmul(ps_k1[:], w_k[:, 0:128], ct_k[:, 512:1024],
                                 start=st, stop=sp)
                nc.tensor.matmul(ps_v0[:], w_k[:, 128:256], ct_k[:, 0:512],
                                 start=st, stop=sp)
                nc.tensor.matmul(ps_v1[:], w_k[:, 128:256], ct_k[:, 512:1024],
                                 start=st, stop=sp)
            # q projection against the resident x.T (overlaps the k/v norm)
            for k in range(KT):
                wq_k = proj_in.tile([128, 512], BF16_DT, tag="wq")
                nc.scalar.dma_start(wq_k[:], wqT[k * 128:(k + 1) * 128, :])
                nc.tensor.matmul(ps_q[:], xT_res[:, k * 128:(k + 1) * 128],
                                 wq_k[:], start=(k == 0), stop=(k == KT - 1))

        # head-replicate the multiplicative mask (GPSIMD elementwise copy)
        m2d = mask_all[:].rearrange("p (s l) -> p s l", l=128)
        for r in range(REP):
            nc.vector.tensor_copy(mask4[:, :, r, :], m2d)

        with nc.named_scope("norm"):
            # copy accumulators out on ACT (idle here); frees proj banks
            qsb = normtmp.tile([128, 512], FP32, tag="qsb")
            nc.scalar.copy(qsb[:], ps_q[:])
            kc = normtmp.tile([128, TNEW], FP32, tag="kc")
            nc.scalar.copy(kc[:, 0:512], ps_k0[:])
            nc.scalar.copy(kc[:, 512:1024], ps_k1[:])
            vsb = normtmp.tile([128, TNEW], BF16_DT, tag="vsb")
            nc.scalar.copy(vsb[:, 0:512], ps_v0[:])
            nc.scalar.copy(vsb[:, 512:1024], ps_v1[:])

            # ---- q rmsnorm + rope (first: unblocks the attention loop) ----
            qsq = normtmp.tile([128, 512], FP32, tag="qsq")
            nc.vector.tensor_mul(qsq[:], qsb[:], qsb[:])
            qsos = normtmp.tile([128, REP], FP32, tag="qsos")
            nc.vector.reduce_sum(
                qsos[:],
                qsq[:].rearrange("p (h l) -> p h l", h=REP),
                axis=mybir.AxisListType.X,
            )
            qrstd = normtmp.tile([128, REP], FP32, tag="qrstd")
            nc.scalar.activation(qrstd[:], qsos[:],
                                 mybir.ActivationFunctionType.Sqrt,
                                 bias=eps_t[:], scale=1.0 / D)
            nc.vector.reciprocal(qrstd[:], qrstd[:])
            qn = normtmp.tile([128, 512], FP32, tag="qn")
            for h in range(REP):
                nc.vector.tensor_scalar_mul(qn[:, h * 128:(h + 1) * 128],
                                            qsb[:, h * 128:(h + 1) * 128],
                                            qrstd[:, h:h + 1])
            qT_all = streams.tile([128, 512], BF16_DT, tag="qT_all")
            qtw = normtmp.tile([128, 512], FP32, tag="qtw")
            for h in range(REP):
                ps_qT = psA.tile([128, 128], FP32, tag="ps_q")
                nc.tensor.transpose(ps_qT[:], qn[:, h * 128:(h + 1) * 128],
                                    ident[:])
                nc.vector.tensor_scalar_mul(qtw[:, h * 128:(h + 1) * 128],
                                            ps_qT[:], qw_t[:])
            qrot = normtmp.tile([128, 512], FP32, tag="qrot")
            nc.sync.dma_start(qrot[0:HALF, :], qtw[HALF:D, :])
            nc.sync.dma_start(qrot[HALF:D, :], qtw[0:HALF, :])
            qa = normtmp.tile([128, 512], FP32, tag="qsq")
            qb = normtmp.tile([128, 512], FP32, tag="qn")
            for h in range(REP):
                sl = slice(h * 128, (h + 1) * 128)
                nc.vector.tensor_mul(qa[:, sl], qtw[:, sl], cosq_t[:])
                nc.vector.tensor_mul(qb[:, sl], qrot[:, sl], sinq_t[:])
            nc.vector.tensor_add(qT_all[:], qa[:], qb[:])

            # ---- k rmsnorm (mean already folded into weights) + rope ----
            ksq = normtmp.tile([128, TNEW], FP32, tag="ksq")
            nc.vector.tensor_mul(ksq[:, 0:512], kc[:, 0:512], kc[:, 0:512])
            nc.vector.tensor_mul(ksq[:, 512:1024], kc[:, 512:1024],
                                 kc[:, 512:1024])
            ps_sos0 = psA.tile([1, 512], FP32, tag="ps_k0")
            ps_sos1 = psA.tile([1, 512], FP32, tag="ps_k1")
            nc.tensor.matmul(ps_sos0[:], ones_col[:], ksq[:, 0:512])
            nc.tensor.matmul(ps_sos1[:], ones_col[:], ksq[:, 512:1024])
            krstd = normtmp.tile([1, TNEW], FP32, tag="krstd")
            nc.scalar.activation(krstd[:, 0:512], ps_sos0[:],
                                 mybir.ActivationFunctionType.Sqrt,
                                 bias=eps_t[0:1, :], scale=1.0 / D)
            nc.scalar.activation(krstd[:, 512:1024], ps_sos1[:],
                                 mybir.ActivationFunctionType.Sqrt,
                                 bias=eps_t[0:1, :], scale=1.0 / D)
            nc.vector.reciprocal(krstd[:], krstd[:])
            ps_krb0 = psA.tile([128, 512], FP32, tag="ps_k0")
            ps_krb1 = psA.tile([128, 512], FP32, tag="ps_k1")
            nc.tensor.matmul(ps_krb0[:], ones_row[:], krstd[:, 0:512])
            nc.tensor.matmul(ps_krb1[:], ones_row[:], krstd[:, 512:1024])
            knw = normtmp.tile([128, TNEW], FP32, tag="knw")
            nc.vector.scalar_tensor_tensor(knw[:, 0:512], kc[:, 0:512],
                                           kw_t[:], ps_krb0[:],
                                           op0=mybir.AluOpType.mult,
                                           op1=mybir.AluOpType.mult)
            nc.vector.scalar_tensor_tensor(knw[:, 512:1024], kc[:, 512:1024],
                                           kw_t[:], ps_krb1[:],
                                           op0=mybir.AluOpType.mult,
                                           op1=mybir.AluOpType.mult)
            krot = normtmp.tile([128, TNEW], FP32, tag="krot")
            nc.sync.dma_start(krot[0:HALF, :], knw[HALF:D, :])
            nc.sync.dma_start(krot[HALF:D, :], knw[0:HALF, :])
            ka = normtmp.tile([128, TNEW], FP32, tag="ksq")
            nc.vector.tensor_mul(ka[:], knw[:], cosk_t[:])
            kb = normtmp.tile([128, TNEW], FP32, tag="kb")
            nc.vector.tensor_mul(kb[:], krot[:], sink_t[:])
            nc.vector.tensor_add(kts[:, SOLD:S], ka[:], kb[:])

            # ---- v transpose into stream tiles via PE ----
            for i in range(TNEW // 128):
                ps_vT = psA.tile([128, 128], BF16_DT, tag="ps_v1")
                nc.tensor.transpose(ps_vT[:], vsb[:, i * 128:(i + 1) * 128],
                                    identb[:])
                nc.vector.tensor_copy(
                    vt[:, SOLD + i * 128:SOLD + (i + 1) * 128], ps_vT[:])

        # ---------------- attention s-loop ----------------
        ps_o = psA.tile([128, 512], FP32, tag="ps_v0")
        ps_sum = psA.tile([1, 512], FP32, tag="ps_q")
        with nc.named_scope("sloop"):
            for s in range(ST):
                if s % 4 == 0:  # wo (host-packed): 8 contiguous 512KB chunks
                    j = s // 4
                    nc.sync.dma_start(wo_res[:, j, :, :], woP[:, j, :, :])
                ps_sc = psS.tile([128, 512], FP32, tag="ps_sc")
                nc.tensor.matmul(ps_sc[:], kts[:, s * 128:(s + 1) * 128],
                                 qT_all[:])
                scb = sloop.tile([128, 512], FP32, tag="scb")
                nc.vector.tensor_copy(scb[:], ps_sc[:])
                exr = sloop.tile([128, 512], BF16_DT, tag="exr")
                nc.scalar.activation(exr[:], scb[:],
                                     mybir.ActivationFunctionType.Exp)
                ex = sloop.tile([128, 512], BF16_DT, tag="ex")
                nc.vector.tensor_mul(
                    ex[:], exr[:],
                    mask4[:, s, :, :].rearrange("p h l -> p (h l)"))
                nc.tensor.matmul(ps_sum[:], ones_colb[:], ex[:],
                                 start=(s == 0), stop=(s == ST - 1))
                nc.tensor.matmul(ps_o[:], vt[:, s * 128:(s + 1) * 128], ex[:],
                                 start=(s == 0), stop=(s == ST - 1))

        # ---------------- normalize ----------------
        with nc.named_scope("fin"):
            rec = normtmp.tile([1, 512], FP32, tag="rec")
            nc.vector.reciprocal(rec[:], ps_sum[:])
            ps_rb = psA.tile([128, 512], FP32, tag="ps_k0")
            nc.tensor.matmul(ps_rb[:], ones_row[:], rec[:])
            osb = normtmp.tile([128, 512], FP32, tag="osb")
            nc.scalar.copy(osb[:], ps_o[:])
            attT = streams.tile([128, 512], BF16_DT, tag="attT")
            nc.vector.tensor_mul(attT[:], osb[:], ps_rb[:])

        # ---------------- output projection (partial) ----------------
        with nc.named_scope("oproj"):
            for e in range(HID // 512):
                ps_y = psA.tile([128, 512], FP32,
                                tag=("ps_k1" if e % 2 else "ps_v0"))
                for h in range(REP):
                    nc.tensor.matmul(
                        ps_y[:], attT[:, h * 128:(h + 1) * 128],
                        wo_res[:, e, h, :],
                        start=(h == 0), stop=(h == REP - 1))
                ysb = sloop.tile([128, 512], FP32, tag="ysb")
                nc.vector.tensor_copy(ysb[:], ps_y[:])
                nc.sync.dma_start(y[:, e * 512:(e + 1) * 512], ysb[:])


def _prepare_inputs(x, x_ctx, cos_q, sin_q, cos_k, sin_k, kv_cache,
                    causal_mask, Wq, Wk, Wv, Wo, q_norm_w, k_norm_w):
    """Host-side sharding/preprocessing. Returns list of per-core in_maps."""
    f32 = np.float32
    x = np.asarray(x, f32)
    x_ctx = np.asarray(x_ctx, f32)
    c = np.concatenate([x_ctx[0], x[0]], axis=0)          # [T, HID]
    cT = np.ascontiguousarray(c.T).astype(BF16)           # [HID, T]

    # x.T packed [p, (k 128l)]: xTp[p, k*128+l] = c.T[k*128+p, T-L+l]
    xTp = np.ascontiguousarray(
        c.T[:, T - L:T].reshape(KT, 128, L).transpose(1, 0, 2)
        .reshape(128, KT * L)).astype(BF16)

    m = np.asarray(causal_mask, f32)[0, 0]                # [L, S]
    # multiplicative mask exp(m), packed [s_local, (s_tile l)]
    maskP = np.ascontiguousarray(np.exp(
        m.T.reshape(S // 128, 128, L).transpose(1, 0, 2).reshape(128, S)))

    cosqT = np.ascontiguousarray(np.asarray(cos_q, f32)[0, 0].T) * SCALE
    sinqT = np.ascontiguousarray(np.asarray(sin_q, f32)[0, 0].T).copy()
    sinqT[:HALF] = -sinqT[:HALF]
    sinqT *= SCALE
    coskT = np.ascontiguousarray(np.asarray(cos_k, f32)[0, 0].T)
    sinkT = np.ascontiguousarray(np.asarray(sin_k, f32)[0, 0].T).copy()
    sinkT[:HALF] = -sinkT[:HALF]

    qwc = np.ascontiguousarray(np.asarray(q_norm_w, f32).reshape(D, 1))
    kwc = np.ascontiguousarray(np.asarray(k_norm_w, f32).reshape(D, 1))

    Wq = np.asarray(Wq, f32)
    Wk = np.asarray(Wk, f32)
    Wv = np.asarray(Wv, f32)
    Wo = np.asarray(Wo, f32)
    kv = np.asarray(kv_cache, f32)

    in_maps = []
    for cidx in range(NCORES):
        hd = slice(cidx * REP * D, (cidx + 1) * REP * D)
        wq_c = Wq[hd].reshape(REP, D, HID)
        wq_c = wq_c - wq_c.mean(axis=1, keepdims=True)    # fold mean-subtract
        wq_c = wq_c.reshape(REP * D, HID)
        wk_c = Wk[cidx * D:(cidx + 1) * D]
        wk_c = wk_c - wk_c.mean(axis=0, keepdims=True)
        wv_c = Wv[cidx * D:(cidx + 1) * D]
        wkvT = np.concatenate([wk_c.T, wv_c.T], axis=1)   # [HID, 256]
        wqTc = np.ascontiguousarray(wq_c.T)               # [HID, 512]
        # wo packed [p, e_chunk, h, 512]: woP[p,j,h,e'] = Wo.T[h*128+p, j*512+e']
        woTc = Wo[:, hd].T.reshape(REP, 128, HID // 512, 512)
        woP = np.ascontiguousarray(woTc.transpose(1, 2, 0, 3))
        ktold = np.ascontiguousarray(kv[0, cidx, T:, :].T)  # [D, SOLD]
        # vold packed [s_local, (tile d)]: voldP[p, n*128+d] = v[n*128+p, d]
        voldP = np.ascontiguousarray(
            kv[1, cidx, T:, :].reshape(SOLD // 128, 128, D)
            .transpose(1, 0, 2).reshape(128, SOLD))
        in_maps.append(dict(
            cT=cT,
            wkvT=np.ascontiguousarray(wkvT).astype(BF16),
            wqT=wqTc.astype(BF16),
            xTp=xTp,
            woP=woP.astype(BF16),
            ktold=ktold.astype(BF16),
            voldP=voldP.astype(BF16),
            identf=np.eye(128, dtype=f32),
            identb2=np.eye(128, dtype=f32).astype(BF16),
            maskT=maskP.astype(BF16),
            cosq=cosqT.astype(f32), sinq=sinqT.astype(f32),
            cosk=coskT.astype(f32), sink=sinkT.astype(f32),
            qw=qwc, kw=kwc,
        ))
    return in_maps


def kernel(**inputs) -> np.ndarray:
    global LAST_RESULTS
    if "nc" not in _PROGRAM_CACHE:
        _PROGRAM_CACHE["nc"] = _build_program()
    nc = _PROGRAM_CACHE["nc"]
    in_maps = _prepare_inputs(**inputs)
    trace = bool(int(os.environ.get("BASS_KERNEL_TRACE", "0")))
    res = run_bass_kernel_spmd(nc, in_maps, list(range(NCORES)), trace=trace)
    LAST_RESULTS = res
    y = np.zeros((L, HID), np.float64)
    for cidx in range(NCORES):
        y += res.results[cidx]["y"].astype(np.float64)
    return y.astype(np.float32).reshape(1, L, HID)



# revision 11
# speedup vs baseline: 1.3860x; 1.3860x over previous
"""Trainium2 Bass kernel for nn_DFlashAttentionSlide (GQA attention block).

Sharding: tensor-parallel over heads across 8 NeuronCores. Core c owns
kv head c and q heads [4c, 4c+4). Activations are replicated; weights /
kv-cache are sharded along the head dim; the output projection is
contraction-sharded, so each core returns a partial [L, HID] output
that the host sums.

v2 layout strategy (per core, all matmuls bf16):
  - q projection first (wq streamed on the sync HWDGE queue), q rmsnorm
    + rope done on [l, (h d)] layout (rotate-half = free-dim slicing;
    SCALE/sign/q_norm_w folded into host tables), then 4 bf16 PE
    transposes into qT_all [d, (h l)].
  - k/v projections as [d, t] (w stationary, cT moving), interleaved on
    the PE queue with the attention s-loop over the 24 CACHED kv tiles
    (which only need q + the cache) so the PE never idles while cT
    streams.
  - s-loop per tile: scores matmul -> exp on ACT directly from PSUM
    (bf16 out) -> VE accumulates sum(exp) into fp32 -> PV matmul
    accumulates into ps_o.  No PSUM->SBUF score copies, no per-tile
    mask (only stream tile 31 is causal-masked), no sum matmuls.
  - rmsnorm rstd = exp(-0.5*ln(meansq)) on ACT: ln and exp live in the
    same activation table set, so the kernel never reloads ACT tables
    (exp is also what softmax needs).  Partition-broadcast of the k
    rstd comes free by using a [128,128] ones stationary in the
    sum-of-squares matmul.
  - attention normalizer 1/sum = exp(-ln(sum)), broadcast the same way.
  - DMA: PE-critical stream (xTp, wq, cT, wkv) on the sync HWDGE queue
    in big chunks; background tensors (cache, tables, mask tile, wo) on
    the scalar HWDGE queue early/late where the ACT engine is idle.
"""

import os
import sys

sys.path.insert(0, "/opt/trn_rl_repo")

import numpy as np
import ml_dtypes

import concourse.bass as bass
import concourse.bacc as bacc
import concourse.tile as tile
from concourse import mybir
from concourse.bass_utils import run_bass_kernel_spmd

BF16 = ml_dtypes.bfloat16

H, HKV, D, HALF = 32, 8, 128, 64
L, T, S, HID = 128, 1024, 4096, 4096
REP = H // HKV          # q heads per kv head (= per core)
EPS = 1e-6
SCALE = D ** -0.5
NCORES = 8
KT = HID // 128         # 32 contraction tiles for projections
ST = S // 128           # 32 s tiles for attention
SOLD = S - T            # 3072 cached stream positions kept
TNEW = T                # 1024 newly projected stream positions

FP32 = mybir.dt.float32
BF16_DT = mybir.dt.bfloat16
AF = mybir.ActivationFunctionType

_PROGRAM_CACHE = {}

# Filled by kernel() when BASS_KERNEL_TRACE=1; read by test.py.
LAST_RESULTS = None


def _build_program():
    nc = bacc.Bacc("TRN2", target_bir_lowering=False, debug=False,
                   num_devices=NCORES)

    # ---- external I/O (per-core values supplied via in_maps) ----
    cT = nc.declare_dram_parameter("cT", [HID, T], BF16_DT, isOutput=False)
    wkvT = nc.declare_dram_parameter("wkvT", [HID, 256], BF16_DT, isOutput=False)
    wqT = nc.declare_dram_parameter("wqT", [HID, 512], BF16_DT, isOutput=False)
    xTp = nc.declare_dram_parameter("xTp", [128, KT * 128], BF16_DT, isOutput=False)
    woP = nc.declare_dram_parameter("woP", [128, HID // 512, REP, 512], BF16_DT, isOutput=False)
    ktold = nc.declare_dram_parameter("ktold", [D, SOLD], BF16_DT, isOutput=False)
    voldP = nc.declare_dram_parameter("voldP", [128, SOLD], BF16_DT, isOutput=False)
    mask31 = nc.declare_dram_parameter("mask31", [128, 512], BF16_DT, isOutput=False)
    cosql = nc.declare_dram_parameter("cosql", [L, 512], FP32, isOutput=False)
    sinql = nc.declare_dram_parameter("sinql", [L, 512], FP32, isOutput=False)
    cosk = nc.declare_dram_parameter("cosk", [D, TNEW], FP32, isOutput=False)
    sink = nc.declare_dram_parameter("sink", [D, TNEW], FP32, isOutput=False)
    kw = nc.declare_dram_parameter("kw", [D, 1], FP32, isOutput=False)
    y = nc.declare_dram_parameter("y", [L, HID], FP32, isOutput=True)

    with tile.TileContext(nc) as tc:
        _emit(nc, tc, cT=cT, wkvT=wkvT, wqT=wqT, xTp=xTp, woP=woP,
              ktold=ktold, voldP=voldP, mask31=mask31,
              cosql=cosql, sinql=sinql, cosk=cosk, sink=sink, kw=kw, y=y)
    nc.compile()
    return nc


def _emit(nc, tc, *, cT, wkvT, wqT, xTp, woP, ktold, voldP, mask31,
          cosql, sinql, cosk, sink, kw, y):
    from contextlib import ExitStack
    from concourse.masks import make_identity

    ctx = ExitStack()
    with ctx:
        # ---------------- pools ----------------
        consts = ctx.enter_context(tc.tile_pool(name="consts", bufs=1))
        streams = ctx.enter_context(tc.tile_pool(name="streams", bufs=1))
        ctp = ctx.enter_context(tc.tile_pool(name="ctp", bufs=1))
        wqp = ctx.enter_context(tc.tile_pool(name="wqp", bufs=1))
        wkvp = ctx.enter_context(tc.tile_pool(name="wkvp", bufs=1))
        ntmp = ctx.enter_context(tc.tile_pool(name="ntmp", bufs=1))
        exp_pool = ctx.enter_context(tc.tile_pool(name="exp", bufs=4))
        ypool = ctx.enter_context(tc.tile_pool(name="ypool", bufs=2))
        wop = ctx.enter_context(tc.tile_pool(name="wop", bufs=1))
        psA = ctx.enter_context(tc.tile_pool(name="psA", bufs=1, space="PSUM"))
        psS = ctx.enter_context(tc.tile_pool(name="psS", bufs=2, space="PSUM"))

        # ---------------- constants (no DMA) ----------------
        ones_full = consts.tile([128, 128], BF16_DT, tag="ones_full")
        nc.vector.memset(ones_full, 1.0)
        identb = consts.tile([128, 128], BF16_DT, tag="identb")
        make_identity(nc, identb[:])
        eps_t = consts.tile([128, 1], FP32, tag="eps")
        nc.vector.memset(eps_t, EPS)

        # ---------------- resident tiles ----------------
        kts = streams.tile([128, S], BF16_DT, tag="kts")
        vt = streams.tile([128, S], BF16_DT, tag="vt")
        xT_res = streams.tile([128, KT * 128], BF16_DT, tag="xT")
        mask31_t = streams.tile([128, 512], BF16_DT, tag="mask31")
        cosql_t = streams.tile([L, 512], FP32, tag="cosql")
        sinql_t = streams.tile([L, 512], FP32, tag="sinql")
        cosk_t = streams.tile([D, TNEW], FP32, tag="cosk")
        sink_t = streams.tile([D, TNEW], FP32, tag="sink")
        kw_t = consts.tile([D, 1], FP32, tag="kw")
        exacc = streams.tile([128, 512], FP32, tag="exacc")
        qT_all = streams.tile([128, 512], BF16_DT, tag="qT_all")

        # -------- scalar (ACT) HWDGE queue: background loads, early ----
        # xTp halves lead (unblock the very first q matmuls), then
        # wkv chunks 0..1 + ct chunks 0..3 (kv tiles 0..7 in phase Q)
        wkv_chunks = [wkvp.tile([128, 4, 256], BF16_DT, tag=f"wkv{w % 2}",
                                name=f"wkvc{w}") for w in range(8)]
        wo_chunks = [wop.tile([128, REP, 512], BF16_DT, tag=f"wo{e % 4}",
                              name=f"woc{e}") for e in range(8)]
        ct_chunks = [ctp.tile([128, 2, T], BF16_DT, tag=f"ct{c % 4}",
                              name=f"ctc{c}") for c in range(16)]
        wkv_src = wkvT[:].rearrange("(w f p) n -> w p f n", w=8, f=4)
        ct_src = cT[:].rearrange("(c i p) t -> c p i t", c=16, i=2)
        nc.scalar.dma_start(xT_res[:, 0:2048], xTp[:, 0:2048])
        nc.scalar.dma_start(xT_res[:, 2048:4096], xTp[:, 2048:4096])
        for w in range(2):
            nc.scalar.dma_start(
                wkv_chunks[w][:], wkv_src[w])
        for c in range(4):
            nc.scalar.dma_start(
                ct_chunks[c][:], ct_src[c])
        nc.scalar.dma_start(kts[:, 0:1536], ktold[:, 0:1536])
        nc.scalar.dma_start(kts[:, 1536:3072], ktold[:, 1536:3072])
        nc.scalar.dma_start(vt[:, 0:1536], voldP[:, 0:1536])
        nc.scalar.dma_start(vt[:, 1536:3072], voldP[:, 1536:3072])
        nc.scalar.dma_start(cosql_t[:], cosql[:])
        nc.scalar.dma_start(sinql_t[:], sinql[:])
        nc.scalar.dma_start(kw_t[:], kw[:])
        nc.scalar.dma_start(cosk_t[:], cosk[:])
        nc.scalar.dma_start(sink_t[:], sink[:])
        nc.scalar.dma_start(mask31_t[:], mask31[:])

        # ---------------- PSUM accumulators ----------------
        ps_q = psA.tile([128, 512], FP32, tag="q")
        ps_k0 = psA.tile([128, 512], FP32, tag="k0")
        ps_k1 = psA.tile([128, 512], FP32, tag="k1")
        ps_v0 = psA.tile([128, 512], FP32, tag="v0")
        ps_v1 = psA.tile([128, 512], FP32, tag="v1")
        ps_o = psA.tile([128, 512], FP32, tag="o")

        def kv_tile(j):
            ctc = ct_chunks[j // 2]
            wc = wkv_chunks[j // 4]
            st = (j == 0)
            sp = (j == KT - 1)
            mov0 = ctc[:, j % 2, 0:512]
            mov1 = ctc[:, j % 2, 512:1024]
            wk = wc[:, j % 4, 0:128]
            wv = wc[:, j % 4, 128:256]
            nc.tensor.matmul(ps_k0[:], wk, mov0, start=st, stop=sp)
            nc.tensor.matmul(ps_k1[:], wk, mov1, start=st, stop=sp)
            nc.tensor.matmul(ps_v0[:], wv, mov0, start=st, stop=sp)
            nc.tensor.matmul(ps_v1[:], wv, mov1, start=st, stop=sp)

        # ---------------- s-loop helpers ----------------
        pending = []  # (s, ex_tile) with sc+exp emitted, pv not yet

        def sc_tile(s):
            ps_sc = psS.tile([128, 512], FP32, tag="sc")
            nc.tensor.matmul(ps_sc[:], kts[:, s * 128:(s + 1) * 128],
                             qT_all[:])
            ex = exp_pool.tile([128, 512], BF16_DT, tag="ex")
            if s == ST - 1:
                exr = ntmp.tile([128, 512], BF16_DT, tag="exr31")
                nc.scalar.activation(exr[:], ps_sc[:], AF.Exp)
                nc.vector.tensor_mul(ex[:], exr[:], mask31_t[:])
            else:
                nc.scalar.activation(ex[:], ps_sc[:], AF.Exp)
            if s == 0:
                nc.vector.tensor_copy(exacc[:], ex[:])
            else:
                nc.vector.tensor_add(exacc[:], exacc[:], ex[:])
            pending.append((s, ex))

        def pv_tile():
            s, ex = pending.pop(0)
            nc.tensor.matmul(ps_o[:], vt[:, s * 128:(s + 1) * 128], ex[:],
                             start=(s == 0), stop=(s == ST - 1))

        # ================= PHASE Q: q projection =================
        with nc.named_scope("qproj"):
            wq_chunks = [wqp.tile([128, 4, 512], BF16_DT, tag=f"wq{u % 3}",
                                  name=f"wqc{u}") for u in range(8)]
            wq_src = wqT[:].rearrange("(u f p) n -> u p f n", u=8, f=4)
            NKVQ = 7   # kv tiles interleaved into the q phase
            for k in range(KT):
                if k % 4 == 0:
                    nc.sync.dma_start(
                        wq_chunks[k // 4][:], wq_src[k // 4])
                nc.tensor.matmul(ps_q[:], xT_res[:, k * 128:(k + 1) * 128],
                                 wq_chunks[k // 4][:, k % 4, :],
                                 start=(k == 0), stop=(k == KT - 1))
                if k % 4 == 3 and k // 4 < NKVQ:
                    kv_tile(k // 4)

        # ================= q rmsnorm + rope (on [l, (h d)]) ============
        with nc.named_scope("qnorm"):
            qsb = ntmp.tile([128, 512], FP32, tag="qsb")
            nc.scalar.copy(qsb[:], ps_q[:])
            qsq = ntmp.tile([128, 512], BF16_DT, tag="qsq")
            nc.scalar.activation(qsq[:], qsb[:], AF.Square)
            qsos = ntmp.tile([128, REP], FP32, tag="qsos")
            nc.vector.reduce_sum(
                qsos[:], qsq[:].rearrange("p (h d) -> p h d", h=REP),
                axis=mybir.AxisListType.X)
            qln = ntmp.tile([128, REP], FP32, tag="qln")
            nc.scalar.activation(qln[:], qsos[:], AF.Ln,
                                 bias=eps_t[:], scale=1.0 / D)
            qrstd = ntmp.tile([128, REP], FP32, tag="qrstd")
            nc.scalar.activation(qrstd[:], qln[:], AF.Exp, scale=-0.5)
            qn = ntmp.tile([128, 512], FP32, tag="qn")
            for h in range(REP):
                nc.vector.tensor_scalar_mul(qn[:, h * 128:(h + 1) * 128],
                                            qsb[:, h * 128:(h + 1) * 128],
                                            qrstd[:, h:h + 1])
            # rope: rotate-half via free-dim slicing; sign/SCALE/w in tables
            qa = ntmp.tile([128, 512], FP32, tag="qsb")
            nc.vector.tensor_mul(qa[:], qn[:], cosql_t[:])
            qn3 = qn[:].rearrange("p (h two d) -> p h two d", h=REP, two=2)
            qa3 = qa[:].rearrange("p (h two d) -> p h two d", h=REP, two=2)
            qb = ntmp.tile([128, 512], FP32, tag="qsq")
            qb3 = qb[:].rearrange("p (h two d) -> p h two d", h=REP, two=2)
            sin3 = sinql_t[:].rearrange("p (h two d) -> p h two d", h=REP, two=2)
            nc.vector.tensor_mul(qb3[:, :, 0, :], qn3[:, :, 1, :],
                                 sin3[:, :, 0, :])
            nc.vector.tensor_mul(qb3[:, :, 1, :], qn3[:, :, 0, :],
                                 sin3[:, :, 1, :])
            qTb = ntmp.tile([128, 512], BF16_DT, tag="qTb")
            nc.vector.tensor_add(qTb[:], qa[:], qb[:])
            for h in range(REP):
                ps_qT = psS.tile([128, 128], BF16_DT, tag="sc")
                nc.tensor.transpose(ps_qT[:], qTb[:, h * 128:(h + 1) * 128],
                                    identb[:])
                nc.vector.tensor_copy(qT_all[:, h * 128:(h + 1) * 128],
                                      ps_qT[:])

        # ================= MAIN: kv proj + cached s-loop ==============
        with nc.named_scope("main"):
            for j in range(NKVQ, KT):
                w = (j + 1) // 4   # prefetch wkv one kv tile ahead
                if (j + 1) % 4 == 0 and 2 <= w < 8:
                    nc.sync.dma_start(
                        wkv_chunks[w][:], wkv_src[w])
                c = j // 2 + 1   # prefetch one ct chunk ahead
                if j % 2 == (NKVQ % 2) and 4 <= c < 16:
                    nc.sync.dma_start(
                        ct_chunks[c][:], ct_src[c])
                if j >= 24 and j % 2 == 0:
                    e = (j - 24) // 2
                    nc.scalar.dma_start(wo_chunks[e][:], woP[:, e, :, :])
                kv_tile(j)
                s = j - NKVQ
                if s <= 19:
                    sc_tile(s)
                    if len(pending) > 2:
                        pv_tile()

        # ================= k/v rmsnorm + rope + new s-tiles ===========
        with nc.named_scope("knorm"):
            # ACT: evacuate v, square k (frees v banks, reads k banks)
            vsb = ntmp.tile([128, TNEW], BF16_DT, tag="vsb")
            nc.scalar.copy(vsb[:, 0:512], ps_v0[:])
            nc.scalar.copy(vsb[:, 512:1024], ps_v1[:])
            ksq = ntmp.tile([128, TNEW], BF16_DT, tag="ksq")
            nc.scalar.activation(ksq[:, 0:512], ps_k0[:], AF.Square)
            nc.scalar.activation(ksq[:, 512:1024], ps_k1[:], AF.Square)

            sc_tile(20); pv_tile()
            sc_tile(21); pv_tile()

            # sum-of-squares with built-in partition broadcast
            ps_sos0 = psA.tile([128, 512], FP32, tag="v0")
            ps_sos1 = psA.tile([128, 512], FP32, tag="v1")
            nc.tensor.matmul(ps_sos0[:], ones_full[:], ksq[:, 0:512])
            nc.tensor.matmul(ps_sos1[:], ones_full[:], ksq[:, 512:1024])
            lns = ntmp.tile([128, 512], FP32, tag="lns")
            krstd = ntmp.tile([128, TNEW], FP32, tag="krstd")
            nc.scalar.activation(lns[:], ps_sos0[:], AF.Ln,
                                 bias=eps_t[:], scale=1.0 / D)
            nc.scalar.activation(krstd[:, 0:512], lns[:], AF.Exp,
                                 scale=-0.5)
            nc.scalar.activation(lns[:], ps_sos1[:], AF.Ln,
                                 bias=eps_t[:], scale=1.0 / D)
            nc.scalar.activation(krstd[:, 512:1024], lns[:],
                                 AF.Exp, scale=-0.5)

            sc_tile(22); pv_tile()
            sc_tile(23); pv_tile(); pv_tile(); pv_tile()

            for e in range(4, 8):
                nc.scalar.dma_start(wo_chunks[e][:], woP[:, e, :, :])

            # knw = (k * kw) * rstd  (reads PSUM k banks, frees them)
            knw = ntmp.tile([128, TNEW], FP32, tag="knw")
            nc.vector.scalar_tensor_tensor(knw[:, 0:512], ps_k0[:], kw_t[:],
                                           krstd[:, 0:512],
                                           op0=mybir.AluOpType.mult,
                                           op1=mybir.AluOpType.mult)
            nc.vector.scalar_tensor_tensor(knw[:, 512:1024], ps_k1[:],
                                           kw_t[:], krstd[:, 512:1024],
                                           op0=mybir.AluOpType.mult,
                                           op1=mybir.AluOpType.mult)
            # rotate-half across partitions (SBUF->SBUF DMA), by halves
            krot = ntmp.tile([128, TNEW], FP32, tag="krot")
            for half in range(2):
                cs = slice(half * 512, (half + 1) * 512)
                nc.sync.dma_start(krot[0:HALF, cs], knw[HALF:D, cs])
                nc.sync.dma_start(krot[HALF:D, cs], knw[0:HALF, cs])
                nc.vector.tensor_mul(knw[:, cs], knw[:, cs], cosk_t[:, cs])
                nc.vector.tensor_mul(krot[:, cs], krot[:, cs], sink_t[:, cs])
                nc.vector.tensor_add(kts[:, SOLD + half * 512:
                                         SOLD + (half + 1) * 512],
                                     knw[:, cs], krot[:, cs])
                # v transposes for this half's 4 stream tiles
                for i in range(4):
                    col = half * 512 + i * 128
                    ps_vT = psS.tile([128, 128], BF16_DT, tag="sc")
                    nc.tensor.transpose(ps_vT[:], vsb[:, col:col + 128],
                                        identb[:])
                    nc.vector.tensor_copy(
                        vt[:, SOLD + col:SOLD + col + 128], ps_vT[:])
                for i in range(4):
                    s = 24 + half * 4 + i
                    sc_tile(s)
                    if len(pending) > 2:
                        pv_tile()
            while pending:
                pv_tile()

        # ================= normalize + output projection ==============
        with nc.named_scope("fin"):
            exacc_b = ntmp.tile([128, 512], BF16_DT, tag="exacc_b")
            nc.vector.tensor_copy(exacc_b[:], exacc[:])
            ps_sum = psA.tile([128, 512], FP32, tag="k0")
            nc.tensor.matmul(ps_sum[:], ones_full[:], exacc_b[:])
            lsum = ntmp.tile([128, 512], FP32, tag="lsum")
            nc.scalar.activation(lsum[:], ps_sum[:], AF.Ln)
            rec = ntmp.tile([128, 512], FP32, tag="rec")
            nc.scalar.activation(rec[:], lsum[:], AF.Exp, scale=-1.0)
            attT = ntmp.tile([128, 512], BF16_DT, tag="attT")
            nc.vector.tensor_mul(attT[:], ps_o[:], rec[:])

        with nc.named_scope("oproj"):
            for e in range(HID // 512):
                ps_y = psA.tile([128, 512], FP32,
                                tag=("k1" if e % 2 else "q"))
                for h in range(REP):
                    nc.tensor.matmul(
                        ps_y[:], attT[:, h * 128:(h + 1) * 128],
                        wo_chunks[e][:, h, :],
                        start=(h == 0), stop=(h == REP - 1))
                ysb = ypool.tile([128, 512], FP32, tag="ysb")
                nc.vector.tensor_copy(ysb[:], ps_y[:])
                nc.sync.dma_start(y[:, e * 512:(e + 1) * 512], ysb[:])


def _prepare_inputs(x, x_ctx, cos_q, sin_q, cos_k, sin_k, kv_cache,
                    causal_mask, Wq, Wk, Wv, Wo, q_norm_w, k_norm_w):
    """Host-side sharding/preprocessing. Returns list of per-core in_maps."""
    f32 = np.float32
    x = np.asarray(x, f32)
    x_ctx = np.asarray(x_ctx, f32)
    c = np.concatenate([x_ctx[0], x[0]], axis=0)          # [T, HID]
    cTm = np.ascontiguousarray(c.T).astype(BF16)          # [HID, T]

    # x.T packed [p, (k l)]: xTp[p, k*128+l] = c.T[k*128+p, T-L+l]
    xTp = np.ascontiguousarray(
        c.T[:, T - L:T].reshape(KT, 128, L).transpose(1, 0, 2)
        .reshape(128, KT * L)).astype(BF16)

    m = np.asarray(causal_mask, f32)[0, 0]                # [L, S]
    # multiplicative mask for stream tile 31 only, head-replicated:
    # mask31[s_local, h*128 + l] = exp(m[l, SOLD+TNEW-128 + s_local])
    m31 = np.exp(m[:, S - 128:S]).T                       # [128, L]
    mask31 = np.ascontiguousarray(
        np.broadcast_to(m31[:, None, :], (128, REP, L)).reshape(128, 512)
    ).astype(BF16)

    qw = np.asarray(q_norm_w, f32).reshape(D)
    kwv = np.asarray(k_norm_w, f32).reshape(D, 1)
    cq = np.asarray(cos_q, f32)[0, 0]                     # [L, D]
    sq = np.asarray(sin_q, f32)[0, 0]
    # q tables on [l, (h d)]: SCALE, rotate-half sign, and q_norm_w folded.
    cosql = cq * qw[None, :] * SCALE                      # [L, D]
    sinql = np.empty((L, D), f32)
    sinql[:, :HALF] = -sq[:, :HALF] * qw[None, HALF:] * SCALE
    sinql[:, HALF:] = sq[:, HALF:] * qw[None, :HALF] * SCALE
    cosql = np.ascontiguousarray(np.tile(cosql, (1, REP)))
    sinql = np.ascontiguousarray(np.tile(sinql, (1, REP)))

    coskT = np.ascontiguousarray(np.asarray(cos_k, f32)[0, 0].T)
    sinkT = np.ascontiguousarray(np.asarray(sin_k, f32)[0, 0].T).copy()
    sinkT[:HALF] = -sinkT[:HALF]

    Wq = np.asarray(Wq, f32)
    Wk = np.asarray(Wk, f32)
    Wv = np.asarray(Wv, f32)
    Wo = np.asarray(Wo, f32)
    kv = np.asarray(kv_cache, f32)

    in_maps = []
    for cidx in range(NCORES):
        hd = slice(cidx * REP * D, (cidx + 1) * REP * D)
        wq_c = Wq[hd].reshape(REP, D, HID)
        wq_c = wq_c - wq_c.mean(axis=1, keepdims=True)    # fold mean-subtract
        wq_c = wq_c.reshape(REP * D, HID)
        wk_c = Wk[cidx * D:(cidx + 1) * D]
        wk_c = wk_c - wk_c.mean(axis=0, keepdims=True)
        wv_c = Wv[cidx * D:(cidx + 1) * D]
        wkvT = np.concatenate([wk_c.T, wv_c.T], axis=1)   # [HID, 256]
        wqTc = np.ascontiguousarray(wq_c.T)               # [HID, 512]
        # wo packed [p, e_chunk, h, 512]: woP[p,j,h,e'] = Wo.T[h*128+p, j*512+e']
        woTc = Wo[:, hd].T.reshape(REP, 128, HID // 512, 512)
        woP = np.ascontiguousarray(woTc.transpose(1, 2, 0, 3))
        ktold = np.ascontiguousarray(kv[0, cidx, T:, :].T)  # [D, SOLD]
        # vold packed [s_local, (tile d)]: voldP[p, n*128+d] = v[n*128+p, d]
        voldP = np.ascontiguousarray(
            kv[1, cidx, T:, :].reshape(SOLD // 128, 128, D)
            .transpose(1, 0, 2).reshape(128, SOLD))
        in_maps.append(dict(
            cT=cTm,
            wkvT=np.ascontiguousarray(wkvT).astype(BF16),
            wqT=wqTc.astype(BF16),
            xTp=xTp,
            woP=woP.astype(BF16),
            ktold=ktold.astype(BF16),
            voldP=voldP.astype(BF16),
            mask31=mask31,
            cosql=cosql, sinql=sinql,
            cosk=coskT, sink=sinkT,
            kw=kwv,
        ))
    return in_maps


def kernel(**inputs) -> np.ndarray:
    global LAST_RESULTS
    if "nc" not in _PROGRAM_CACHE:
        _PROGRAM_CACHE["nc"] = _build_program()
    nc = _PROGRAM_CACHE["nc"]
    in_maps = _prepare_inputs(**inputs)
    trace = bool(int(os.environ.get("BASS_KERNEL_TRACE", "0")))
    res = run_bass_kernel_spmd(nc, in_maps, list(range(NCORES)), trace=trace)
    LAST_RESULTS = res
    y = np.zeros((L, HID), np.float64)
    for cidx in range(NCORES):
        y += res.results[cidx]["y"].astype(np.float64)
    return y.astype(np.float32).reshape(1, L, HID)


# revision 13
# speedup vs baseline: 1.4849x; 1.0714x over previous
"""Trainium2 Bass kernel for nn_DFlashAttentionSlide (GQA attention block).

Sharding: tensor-parallel over heads across 8 NeuronCores. Core c owns
kv head c and q heads [4c, 4c+4). Activations are replicated; weights /
kv-cache are sharded along the head dim; the output projection is
contraction-sharded, so each core returns a partial [L, HID] output
that the host sums.

v2 layout strategy (per core, all matmuls bf16):
  - q projection first (wq streamed on the sync HWDGE queue), q rmsnorm
    + rope done on [l, (h d)] layout (rotate-half = free-dim slicing;
    SCALE/sign/q_norm_w folded into host tables), then 4 bf16 PE
    transposes into qT_all [d, (h l)].
  - k/v projections as [d, t] (w stationary, cT moving), interleaved on
    the PE queue with the attention s-loop over the 24 CACHED kv tiles
    (which only need q + the cache) so the PE never idles while cT
    streams.
  - s-loop per tile: scores matmul -> exp on ACT directly from PSUM
    (bf16 out) -> VE accumulates sum(exp) into fp32 -> PV matmul
    accumulates into ps_o.  No PSUM->SBUF score copies, no per-tile
    mask (only stream tile 31 is causal-masked), no sum matmuls.
  - rmsnorm rstd = exp(-0.5*ln(meansq)) on ACT: ln and exp live in the
    same activation table set, so the kernel never reloads ACT tables
    (exp is also what softmax needs).  Partition-broadcast of the k
    rstd comes free by using a [128,128] ones stationary in the
    sum-of-squares matmul.
  - attention normalizer 1/sum = exp(-ln(sum)), broadcast the same way.
  - DMA: PE-critical stream (xTp, wq, cT, wkv) on the sync HWDGE queue
    in big chunks; background tensors (cache, tables, mask tile, wo) on
    the scalar HWDGE queue early/late where the ACT engine is idle.
"""

import os
import sys

sys.path.insert(0, "/opt/trn_rl_repo")

import numpy as np
import ml_dtypes

import concourse.bass as bass
import concourse.bacc as bacc

# Force every ACT-table choice to the one set containing copy/square/ln/exp
# (natural_log_exp_and_others): the default first-match pass flips between
# per-function tables, costing a 1.28us ACT table reload around every
# rmsnorm.  Entries for all other sets are emptied so the insertion pass
# can only ever pick the superset (its act_func_set_id is preserved).
from concourse.hw_specs import get_activation_tables as _real_act_tables


def _superset_act_tables(arch):
    out = {}
    for name, fns in _real_act_tables(arch).items():
        out[name] = fns if name == "natural_log_exp_and_others" else set()
    return out


bacc.get_activation_tables = _superset_act_tables
import concourse.tile as tile
from concourse import mybir
from concourse.bass_utils import run_bass_kernel_spmd

BF16 = ml_dtypes.bfloat16

H, HKV, D, HALF = 32, 8, 128, 64
L, T, S, HID = 128, 1024, 4096, 4096
REP = H // HKV          # q heads per kv head (= per core)
EPS = 1e-6
SCALE = D ** -0.5
NCORES = 8
KT = HID // 128         # 32 contraction tiles for projections
ST = S // 128           # 32 s tiles for attention
SOLD = S - T            # 3072 cached stream positions kept
TNEW = T                # 1024 newly projected stream positions

FP32 = mybir.dt.float32
BF16_DT = mybir.dt.bfloat16
AF = mybir.ActivationFunctionType

_PROGRAM_CACHE = {}

# Filled by kernel() when BASS_KERNEL_TRACE=1; read by test.py.
LAST_RESULTS = None


def _build_program():
    nc = bacc.Bacc("TRN2", target_bir_lowering=False, debug=False,
                   num_devices=NCORES)

    # ---- external I/O (per-core values supplied via in_maps) ----
    cT = nc.declare_dram_parameter("cT", [HID, T], BF16_DT, isOutput=False)
    wkvT = nc.declare_dram_parameter("wkvT", [HID, 256], BF16_DT, isOutput=False)
    wqT = nc.declare_dram_parameter("wqT", [HID, 512], BF16_DT, isOutput=False)
    xTp = nc.declare_dram_parameter("xTp", [128, KT * 128], BF16_DT, isOutput=False)
    woP = nc.declare_dram_parameter("woP", [128, HID // 512, REP, 512], BF16_DT, isOutput=False)
    ktold = nc.declare_dram_parameter("ktold", [D, SOLD], BF16_DT, isOutput=False)
    voldP = nc.declare_dram_parameter("voldP", [128, SOLD], BF16_DT, isOutput=False)
    mask31 = nc.declare_dram_parameter("mask31", [128, 512], BF16_DT, isOutput=False)
    cosql = nc.declare_dram_parameter("cosql", [L, 512], FP32, isOutput=False)
    sinql = nc.declare_dram_parameter("sinql", [L, 512], FP32, isOutput=False)
    cosk = nc.declare_dram_parameter("cosk", [D, TNEW], FP32, isOutput=False)
    sink = nc.declare_dram_parameter("sink", [D, TNEW], FP32, isOutput=False)
    kw = nc.declare_dram_parameter("kw", [D, 1], FP32, isOutput=False)
    y = nc.declare_dram_parameter("y", [L, HID], FP32, isOutput=True)

    with tile.TileContext(nc) as tc:
        _emit(nc, tc, cT=cT, wkvT=wkvT, wqT=wqT, xTp=xTp, woP=woP,
              ktold=ktold, voldP=voldP, mask31=mask31,
              cosql=cosql, sinql=sinql, cosk=cosk, sink=sink, kw=kw, y=y)
    nc.compile()
    return nc


def _emit(nc, tc, *, cT, wkvT, wqT, xTp, woP, ktold, voldP, mask31,
          cosql, sinql, cosk, sink, kw, y):
    from contextlib import ExitStack
    from concourse.masks import make_identity

    ctx = ExitStack()
    with ctx:
        # ---------------- pools ----------------
        consts = ctx.enter_context(tc.tile_pool(name="consts", bufs=1))
        streams = ctx.enter_context(tc.tile_pool(name="streams", bufs=1))
        ctp = ctx.enter_context(tc.tile_pool(name="ctp", bufs=1))
        wqp = ctx.enter_context(tc.tile_pool(name="wqp", bufs=1))
        wkvp = ctx.enter_context(tc.tile_pool(name="wkvp", bufs=1))
        ntmp = ctx.enter_context(tc.tile_pool(name="ntmp", bufs=1))
        exp_pool = ctx.enter_context(tc.tile_pool(name="exp", bufs=4))
        ypool = ctx.enter_context(tc.tile_pool(name="ypool", bufs=2))
        wop = ctx.enter_context(tc.tile_pool(name="wop", bufs=1))
        psA = ctx.enter_context(tc.tile_pool(name="psA", bufs=1, space="PSUM"))
        psS = ctx.enter_context(tc.tile_pool(name="psS", bufs=2, space="PSUM"))

        # ---------------- constants (no DMA) ----------------
        ones_full = consts.tile([128, 128], BF16_DT, tag="ones_full")
        nc.vector.memset(ones_full, 1.0)
        identb = consts.tile([128, 128], BF16_DT, tag="identb")
        make_identity(nc, identb[:])
        eps_t = consts.tile([128, 1], FP32, tag="eps")
        nc.vector.memset(eps_t, EPS)

        # ---------------- resident tiles ----------------
        kts = streams.tile([128, S], BF16_DT, tag="kts")
        vt = streams.tile([128, S], BF16_DT, tag="vt")
        xT_res = streams.tile([128, KT * 128], BF16_DT, tag="xT")
        mask31_t = streams.tile([128, 512], BF16_DT, tag="mask31")
        cosql_t = streams.tile([L, 512], FP32, tag="cosql")
        sinql_t = streams.tile([L, 512], FP32, tag="sinql")
        cosk_t = streams.tile([D, TNEW], FP32, tag="cosk")
        sink_t = streams.tile([D, TNEW], FP32, tag="sink")
        kw_t = consts.tile([D, 1], FP32, tag="kw")
        exacc = streams.tile([128, 512], FP32, tag="exacc")
        qT_all = streams.tile([128, 512], BF16_DT, tag="qT_all")

        # -------- scalar (ACT) HWDGE queue: background loads, early ----
        # xTp halves lead (unblock the very first q matmuls), then
        # wkv chunks 0..1 + ct chunks 0..3 (kv tiles 0..7 in phase Q)
        wkv_chunks = [wkvp.tile([128, 4, 256], BF16_DT, tag=f"wkv{w % 2}",
                                name=f"wkvc{w}") for w in range(8)]
        wo_chunks = [wop.tile([128, REP, 512], BF16_DT, tag=f"wo{e % 4}",
                              name=f"woc{e}") for e in range(8)]
        ct_chunks = [ctp.tile([128, 2, T], BF16_DT, tag=f"ct{c % 4}",
                              name=f"ctc{c}") for c in range(16)]
        wkv_src = wkvT[:].rearrange("(w f p) n -> w p f n", w=8, f=4)
        ct_src = cT[:].rearrange("(c i p) t -> c p i t", c=16, i=2)
        nc.scalar.dma_start(xT_res[:, 0:2048], xTp[:, 0:2048])
        nc.scalar.dma_start(xT_res[:, 2048:4096], xTp[:, 2048:4096])
        nc.scalar.dma_start(ct_chunks[0][:], ct_src[0])
        nc.scalar.dma_start(wkv_chunks[0][:], wkv_src[0])
        nc.scalar.dma_start(ct_chunks[1][:], ct_src[1])
        nc.scalar.dma_start(kts[:, 0:1536], ktold[:, 0:1536])
        nc.scalar.dma_start(kts[:, 1536:3072], ktold[:, 1536:3072])
        nc.scalar.dma_start(vt[:, 0:1536], voldP[:, 0:1536])
        nc.scalar.dma_start(vt[:, 1536:3072], voldP[:, 1536:3072])
        nc.scalar.dma_start(ct_chunks[2][:], ct_src[2])
        nc.scalar.dma_start(wkv_chunks[1][:], wkv_src[1])
        nc.scalar.dma_start(cosql_t[:], cosql[:])
        nc.scalar.dma_start(sinql_t[:], sinql[:])
        nc.scalar.dma_start(kw_t[:], kw[:])
        nc.scalar.dma_start(cosk_t[:], cosk[:])
        nc.scalar.dma_start(sink_t[:], sink[:])
        nc.scalar.dma_start(mask31_t[:], mask31[:])

        # ---------------- PSUM accumulators ----------------
        ps_q = psA.tile([128, 512], FP32, tag="q")
        ps_k0 = psA.tile([128, 512], FP32, tag="k0")
        ps_k1 = psA.tile([128, 512], FP32, tag="k1")
        ps_v0 = psA.tile([128, 512], FP32, tag="v0")
        ps_v1 = psA.tile([128, 512], FP32, tag="v1")
        ps_o = psA.tile([128, 512], FP32, tag="o")

        def kv_tile(j):
            ctc = ct_chunks[j // 2]
            wc = wkv_chunks[j // 4]
            st = (j == 0)
            sp = (j == KT - 1)
            mov0 = ctc[:, j % 2, 0:512]
            mov1 = ctc[:, j % 2, 512:1024]
            wk = wc[:, j % 4, 0:128]
            wv = wc[:, j % 4, 128:256]
            nc.tensor.matmul(ps_k0[:], wk, mov0, start=st, stop=sp)
            nc.tensor.matmul(ps_k1[:], wk, mov1, start=st, stop=sp)
            nc.tensor.matmul(ps_v0[:], wv, mov0, start=st, stop=sp)
            nc.tensor.matmul(ps_v1[:], wv, mov1, start=st, stop=sp)

        # ---------------- s-loop helpers ----------------
        pending = []  # (s, ex_tile) with sc+exp emitted, pv not yet

        def sc_tile(s):
            ps_sc = psS.tile([128, 512], FP32, tag="sc")
            nc.tensor.matmul(ps_sc[:], kts[:, s * 128:(s + 1) * 128],
                             qT_all[:])
            ex = exp_pool.tile([128, 512], BF16_DT, tag="ex")
            if s == ST - 1:
                exr = ntmp.tile([128, 512], BF16_DT, tag="exr31")
                nc.scalar.activation(exr[:], ps_sc[:], AF.Exp)
                nc.vector.tensor_mul(ex[:], exr[:], mask31_t[:])
            else:
                nc.scalar.activation(ex[:], ps_sc[:], AF.Exp)
            if s == 0:
                nc.vector.tensor_copy(exacc[:], ex[:])
            else:
                nc.vector.tensor_add(exacc[:], exacc[:], ex[:])
            pending.append((s, ex))

        def pv_tile():
            s, ex = pending.pop(0)
            nc.tensor.matmul(ps_o[:], vt[:, s * 128:(s + 1) * 128], ex[:],
                             start=(s == 0), stop=(s == ST - 1))

        # ================= PHASE Q: q projection =================
        with nc.named_scope("qproj"):
            wq_chunks = [wqp.tile([128, 4, 512], BF16_DT, tag=f"wq{u % 3}",
                                  name=f"wqc{u}") for u in range(8)]
            wq_src = wqT[:].rearrange("(u f p) n -> u p f n", u=8, f=4)
            NKVQ = 6   # kv tiles interleaved into the q phase
            for k in range(KT):
                if k % 4 == 0:
                    nc.sync.dma_start(
                        wq_chunks[k // 4][:], wq_src[k // 4])
                nc.tensor.matmul(ps_q[:], xT_res[:, k * 128:(k + 1) * 128],
                                 wq_chunks[k // 4][:, k % 4, :],
                                 start=(k == 0), stop=(k == KT - 1))
                if k % 4 == 3 and 1 <= k // 4 <= NKVQ:
                    kv_tile(k // 4 - 1)

        # ================= q rmsnorm + rope (on [l, (h d)]) ============
        with nc.named_scope("qnorm"):
            qsb = ntmp.tile([128, 512], FP32, tag="qsb")
            nc.scalar.copy(qsb[:], ps_q[:])
            qsq = ntmp.tile([128, 512], BF16_DT, tag="qsq")
            nc.scalar.activation(qsq[:], qsb[:], AF.Square)
            qsos = ntmp.tile([128, REP], FP32, tag="qsos")
            nc.vector.reduce_sum(
                qsos[:], qsq[:].rearrange("p (h d) -> p h d", h=REP),
                axis=mybir.AxisListType.X)
            qln = ntmp.tile([128, REP], FP32, tag="qln")
            nc.scalar.activation(qln[:], qsos[:], AF.Ln,
                                 bias=eps_t[:], scale=1.0 / D)
            qrstd = ntmp.tile([128, REP], FP32, tag="qrstd")
            nc.scalar.activation(qrstd[:], qln[:], AF.Exp, scale=-0.5)
            qn = ntmp.tile([128, 512], FP32, tag="qn")
            for h in range(REP):
                nc.vector.tensor_scalar_mul(qn[:, h * 128:(h + 1) * 128],
                                            qsb[:, h * 128:(h + 1) * 128],
                                            qrstd[:, h:h + 1])
            # rope: rotate-half via free-dim slicing; sign/SCALE/w in tables
            qa = ntmp.tile([128, 512], FP32, tag="qsb")
            nc.vector.tensor_mul(qa[:], qn[:], cosql_t[:])
            qn3 = qn[:].rearrange("p (h two d) -> p h two d", h=REP, two=2)
            qa3 = qa[:].rearrange("p (h two d) -> p h two d", h=REP, two=2)
            qb = ntmp.tile([128, 512], FP32, tag="qsq")
            qb3 = qb[:].rearrange("p (h two d) -> p h two d", h=REP, two=2)
            sin3 = sinql_t[:].rearrange("p (h two d) -> p h two d", h=REP, two=2)
            nc.vector.tensor_mul(qb3[:, :, 0, :], qn3[:, :, 1, :],
                                 sin3[:, :, 0, :])
            nc.vector.tensor_mul(qb3[:, :, 1, :], qn3[:, :, 0, :],
                                 sin3[:, :, 1, :])
            qTb = ntmp.tile([128, 512], BF16_DT, tag="qTb")
            nc.vector.tensor_add(qTb[:], qa[:], qb[:])
            for h in range(REP):
                ps_qT = psS.tile([128, 128], BF16_DT, tag="sc")
                nc.tensor.transpose(ps_qT[:], qTb[:, h * 128:(h + 1) * 128],
                                    identb[:])
                nc.vector.tensor_copy(qT_all[:, h * 128:(h + 1) * 128],
                                      ps_qT[:])

        # ================= MAIN: kv proj + cached s-loop ==============
        with nc.named_scope("main"):
            nc.sync.dma_start(ct_chunks[3][:], ct_src[3])
            for j in range(NKVQ, KT):
                w = (j + 1) // 4   # prefetch wkv one kv tile ahead
                if (j + 1) % 4 == 0 and 2 <= w < 8:
                    nc.sync.dma_start(
                        wkv_chunks[w][:], wkv_src[w])
                c = j // 2 + 1   # prefetch one ct chunk ahead
                if j % 2 == 0 and 4 <= c < 16:
                    nc.sync.dma_start(
                        ct_chunks[c][:], ct_src[c])
                if j >= 24 and j % 2 == 0:
                    e = (j - 24) // 2
                    nc.scalar.dma_start(wo_chunks[e][:], woP[:, e, :, :])
                kv_tile(j)
                s = j - NKVQ
                if s <= 19:
                    sc_tile(s)
                    if len(pending) > 2:
                        pv_tile()

        # ================= k/v rmsnorm + rope + new s-tiles ===========
        with nc.named_scope("knorm"):
            # ACT: evacuate v, square k (frees v banks, reads k banks)
            vsb = ntmp.tile([128, TNEW], BF16_DT, tag="vsb")
            nc.scalar.copy(vsb[:, 0:512], ps_v0[:])
            nc.scalar.copy(vsb[:, 512:1024], ps_v1[:])
            ksq = ntmp.tile([128, TNEW], BF16_DT, tag="ksq")
            nc.scalar.activation(ksq[:, 0:512], ps_k0[:], AF.Square)
            nc.scalar.activation(ksq[:, 512:1024], ps_k1[:], AF.Square)

            sc_tile(20); pv_tile()
            sc_tile(21); pv_tile()

            # sum-of-squares with built-in partition broadcast
            ps_sos0 = psA.tile([128, 512], FP32, tag="v0")
            ps_sos1 = psA.tile([128, 512], FP32, tag="v1")
            nc.tensor.matmul(ps_sos0[:], ones_full[:], ksq[:, 0:512])
            nc.tensor.matmul(ps_sos1[:], ones_full[:], ksq[:, 512:1024])
            lns = ntmp.tile([128, TNEW], FP32, tag="lns")
            krstd = ntmp.tile([128, TNEW], FP32, tag="krstd")
            nc.scalar.activation(lns[:, 0:512], ps_sos0[:], AF.Ln,
                                 bias=eps_t[:], scale=1.0 / D)
            nc.scalar.activation(lns[:, 512:1024], ps_sos1[:], AF.Ln,
                                 bias=eps_t[:], scale=1.0 / D)
            nc.scalar.activation(krstd[:, 0:512], lns[:, 0:512], AF.Exp,
                                 scale=-0.5)
            nc.scalar.activation(krstd[:, 512:1024], lns[:, 512:1024],
                                 AF.Exp, scale=-0.5)

            sc_tile(22); pv_tile()
            sc_tile(23); pv_tile(); pv_tile(); pv_tile()

            for e in range(4, 8):
                nc.scalar.dma_start(wo_chunks[e][:], woP[:, e, :, :])

            # knw = (k * kw) * rstd  (reads PSUM k banks, frees them)
            knw = ntmp.tile([128, TNEW], FP32, tag="knw")
            nc.vector.scalar_tensor_tensor(knw[:, 0:512], ps_k0[:], kw_t[:],
                                           krstd[:, 0:512],
                                           op0=mybir.AluOpType.mult,
                                           op1=mybir.AluOpType.mult)
            nc.vector.scalar_tensor_tensor(knw[:, 512:1024], ps_k1[:],
                                           kw_t[:], krstd[:, 512:1024],
                                           op0=mybir.AluOpType.mult,
                                           op1=mybir.AluOpType.mult)
            # rotate-half across partitions (SBUF->SBUF DMA), by halves
            krot = ntmp.tile([128, TNEW], FP32, tag="krot")
            for half in range(2):
                cs = slice(half * 512, (half + 1) * 512)
                nc.sync.dma_start(krot[0:HALF, cs], knw[HALF:D, cs])
                nc.sync.dma_start(krot[HALF:D, cs], knw[0:HALF, cs])
                nc.vector.tensor_mul(knw[:, cs], knw[:, cs], cosk_t[:, cs])
                nc.vector.tensor_mul(krot[:, cs], krot[:, cs], sink_t[:, cs])
                nc.vector.tensor_add(kts[:, SOLD + half * 512:
                                         SOLD + (half + 1) * 512],
                                     knw[:, cs], krot[:, cs])
                # v transposes for this half's 4 stream tiles
                for i in range(4):
                    col = half * 512 + i * 128
                    ps_vT = psS.tile([128, 128], BF16_DT, tag="sc")
                    nc.tensor.transpose(ps_vT[:], vsb[:, col:col + 128],
                                        identb[:])
                    nc.vector.tensor_copy(
                        vt[:, SOLD + col:SOLD + col + 128], ps_vT[:])
                for i in range(4):
                    s = 24 + half * 4 + i
                    sc_tile(s)
                    if len(pending) > 2:
                        pv_tile()
            while pending:
                pv_tile()

        # ================= normalize + output projection ==============
        with nc.named_scope("fin"):
            exacc_b = ntmp.tile([128, 512], BF16_DT, tag="exacc_b")
            nc.vector.tensor_copy(exacc_b[:], exacc[:])
            ps_sum = psA.tile([128, 512], FP32, tag="k0")
            nc.tensor.matmul(ps_sum[:], ones_full[:], exacc_b[:])
            rec = ntmp.tile([128, 512], FP32, tag="rec")
            nc.vector.reciprocal_approx_fast(out=rec[:], in_=ps_sum[:])
            attT = ntmp.tile([128, 512], BF16_DT, tag="attT")
            for h in range(REP):
                hs = slice(h * 128, (h + 1) * 128)
                nc.vector.tensor_mul(attT[:, hs], ps_o[:, hs], rec[:, hs])

        with nc.named_scope("oproj"):
            for e in range(HID // 512):
                ps_y = psA.tile([128, 512], FP32,
                                tag=("k1" if e % 2 else "q"))
                for h in range(REP):
                    nc.tensor.matmul(
                        ps_y[:], attT[:, h * 128:(h + 1) * 128],
                        wo_chunks[e][:, h, :],
                        start=(h == 0), stop=(h == REP - 1))
                ysb = ypool.tile([128, 512], FP32, tag="ysb")
                nc.vector.tensor_copy(ysb[:], ps_y[:])
                nc.sync.dma_start(y[:, e * 512:(e + 1) * 512], ysb[:])


def _prepare_inputs(x, x_ctx, cos_q, sin_q, cos_k, sin_k, kv_cache,
                    causal_mask, Wq, Wk, Wv, Wo, q_norm_w, k_norm_w):
    """Host-side sharding/preprocessing. Returns list of per-core in_maps."""
    f32 = np.float32
    x = np.asarray(x, f32)
    x_ctx = np.asarray(x_ctx, f32)
    c = np.concatenate([x_ctx[0], x[0]], axis=0)          # [T, HID]
    cTm = np.ascontiguousarray(c.T).astype(BF16)          # [HID, T]

    # x.T packed [p, (k l)]: xTp[p, k*128+l] = c.T[k*128+p, T-L+l]
    xTp = np.ascontiguousarray(
        c.T[:, T - L:T].reshape(KT, 128, L).transpose(1, 0, 2)
        .reshape(128, KT * L)).astype(BF16)

    m = np.asarray(causal_mask, f32)[0, 0]                # [L, S]
    # multiplicative mask for stream tile 31 only, head-replicated:
    # mask31[s_local, h*128 + l] = exp(m[l, SOLD+TNEW-128 + s_local])
    m31 = np.exp(m[:, S - 128:S]).T                       # [128, L]
    mask31 = np.ascontiguousarray(
        np.broadcast_to(m31[:, None, :], (128, REP, L)).reshape(128, 512)
    ).astype(BF16)

    qw = np.asarray(q_norm_w, f32).reshape(D)
    kwv = np.asarray(k_norm_w, f32).reshape(D, 1)
    cq = np.asarray(cos_q, f32)[0, 0]                     # [L, D]
    sq = np.asarray(sin_q, f32)[0, 0]
    # q tables on [l, (h d)]: SCALE, rotate-half sign, and q_norm_w folded.
    cosql = cq * qw[None, :] * SCALE                      # [L, D]
    sinql = np.empty((L, D), f32)
    sinql[:, :HALF] = -sq[:, :HALF] * qw[None, HALF:] * SCALE
    sinql[:, HALF:] = sq[:, HALF:] * qw[None, :HALF] * SCALE
    cosql = np.ascontiguousarray(np.tile(cosql, (1, REP)))
    sinql = np.ascontiguousarray(np.tile(sinql, (1, REP)))

    coskT = np.ascontiguousarray(np.asarray(cos_k, f32)[0, 0].T)
    sinkT = np.ascontiguousarray(np.asarray(sin_k, f32)[0, 0].T).copy()
    sinkT[:HALF] = -sinkT[:HALF]

    Wq = np.asarray(Wq, f32)
    Wk = np.asarray(Wk, f32)
    Wv = np.asarray(Wv, f32)
    Wo = np.asarray(Wo, f32)
    kv = np.asarray(kv_cache, f32)

    in_maps = []
    for cidx in range(NCORES):
        hd = slice(cidx * REP * D, (cidx + 1) * REP * D)
        wq_c = Wq[hd].reshape(REP, D, HID)
        wq_c = wq_c - wq_c.mean(axis=1, keepdims=True)    # fold mean-subtract
        wq_c = wq_c.reshape(REP * D, HID)
        wk_c = Wk[cidx * D:(cidx + 1) * D]
        wk_c = wk_c - wk_c.mean(axis=0, keepdims=True)
        wv_c = Wv[cidx * D:(cidx + 1) * D]
        wkvT = np.concatenate([wk_c.T, wv_c.T], axis=1)   # [HID, 256]
        wqTc = np.ascontiguousarray(wq_c.T)               # [HID, 512]
        # wo packed [p, e_chunk, h, 512]: woP[p,j,h,e'] = Wo.T[h*128+p, j*512+e']
        woTc = Wo[:, hd].T.reshape(REP, 128, HID // 512, 512)
        woP = np.ascontiguousarray(woTc.transpose(1, 2, 0, 3))
        ktold = np.ascontiguousarray(kv[0, cidx, T:, :].T)  # [D, SOLD]
        # vold packed [s_local, (tile d)]: voldP[p, n*128+d] = v[n*128+p, d]
        voldP = np.ascontiguousarray(
            kv[1, cidx, T:, :].reshape(SOLD // 128, 128, D)
            .transpose(1, 0, 2).reshape(128, SOLD))
        in_maps.append(dict(
            cT=cTm,
            wkvT=np.ascontiguousarray(wkvT).astype(BF16),
            wqT=wqTc.astype(BF16),
            xTp=xTp,
            woP=woP.astype(BF16),
            ktold=ktold.astype(BF16),
            voldP=voldP.astype(BF16),
            mask31=mask31,
            cosql=cosql, sinql=sinql,
            cosk=coskT, sink=sinkT,
            kw=kwv,
        ))
    return in_maps


def kernel(**inputs) -> np.ndarray:
    global LAST_RESULTS
    if "nc" not in _PROGRAM_CACHE:
        _PROGRAM_CACHE["nc"] = _build_program()
    nc = _PROGRAM_CACHE["nc"]
    in_maps = _prepare_inputs(**inputs)
    trace = bool(int(os.environ.get("BASS_KERNEL_TRACE", "0")))
    res = run_bass_kernel_spmd(nc, in_maps, list(range(NCORES)), trace=trace)
    LAST_RESULTS = res
    y = np.zeros((L, HID), np.float64)
    for cidx in range(NCORES):
        y += res.results[cidx]["y"].astype(np.float64)
    return y.astype(np.float32).reshape(1, L, HID)
